# revision 39
# baseline (speedup 1.0000x reference)
"""Expert-parallel MoE kernel for one TRN2 chip (8 NeuronCores).

nn_DynamicRouterMoE: B=4, T=2048, C=1024, E=16, H=4096, top-2 routing.

v3: router/top-2/softmax/dispatch on the HOST (fp64 -> exact ordering vs
the fp32 reference; min top-2 logit gap ~1e-5 >> fp64 error). The device
runs a pure FFN per core over SLOTS of host-compacted fp16 token panels.

Load balance under the SPMD constraint (all cores run one program, so
panel capacities are static): each expert's token list is split into two
groups; the 32 groups are binned into 4 "bands" of 8 (one group per core
per band). Band capacities come from a small search minimizing the total
(2080 slots/core vs 2048 ideal vs 2176 for whole-expert pairing).

Per core, per slot s (smallest band first for a short input prologue):
  xg[s]: [128(c), CC, cap_s] fp16 panel (host-gathered, transposed)
  for hc in 8 chunks of HC=512 over H (w1/w2 streamed, 2 MB/chunk):
    hT = relu(xg @ w1_chunk + b1)   (PE fp16 -> PSUM, Scalar relu)
    yT += hT @ w2_chunk             (PE fp16, Vector accumulate fp32)
  yT -> HBM raw (channel-major); host adds b2, gates, scatter-adds.

PE roofline: 2080 slots x 512 MAC-cycles @2.4 GHz + NX floor ~= 451 us.
"""

from contextlib import ExitStack
from itertools import combinations_with_replacement

import numpy as np

import concourse.bacc as bacc
import concourse.mybir as mybir
from concourse import bass_utils
from concourse.tile import TileContext

dt = mybir.dt
AF = mybir.ActivationFunctionType

# problem shape (hardcoded per contest contract)
B, T, C, E, H = 4, 2048, 1024, 16, 4096
N = B * T                  # 8192 tokens
NCORES = 8
NBANDS = 4                 # slots (token panels) per core
HC = 512                   # H chunk streamed from HBM
CC = C // 128              # 8 contraction chunks
NHC = H // HC              # 8 H chunks
HT = HC // 128             # 4
MOVW = 512                 # moving-operand tile width (tokens per matmul)

_NC_CACHE = {}
_LAST_META = {}


def _build(caps):
    """caps: ascending static token capacities of the NBANDS slots."""
    nc = bacc.Bacc("TRN2", target_bir_lowering=False, debug=False,
                   num_devices=NCORES)
    xgd = [nc.dram_tensor(f"xg{s}", [128, CC * cap], dt.float16,
                          kind="ExternalInput") for s, cap in enumerate(caps)]
    w1 = nc.dram_tensor("w1", [NBANDS, NHC, HT, 128, CC * 128], dt.float16,
                        kind="ExternalInput")
    w2 = nc.dram_tensor("w2", [NBANDS, NHC, 128, HT * C], dt.float16,
                        kind="ExternalInput")
    b1 = nc.dram_tensor("b1", [NBANDS, 128, H // 128], dt.float32,
                        kind="ExternalInput")
    ytd = [nc.dram_tensor(f"yt{s}", [CC, 128, cap], dt.float32,
                          kind="ExternalOutput") for s, cap in enumerate(caps)]

    with TileContext(nc) as tc, ExitStack() as ctx:
        const_pool = ctx.enter_context(tc.tile_pool(name="const", bufs=1))
        xg_pool = ctx.enter_context(tc.tile_pool(name="xg", bufs=1))
        w_pool = ctx.enter_context(tc.tile_pool(name="w", bufs=2))
        h_pool = ctx.enter_context(tc.tile_pool(name="h", bufs=2))
        yacc_pool = ctx.enter_context(tc.tile_pool(name="yacc", bufs=1))
        psh_pool = ctx.enter_context(tc.tile_pool(name="psh", bufs=4, space="PSUM"))
        psy_pool = ctx.enter_context(tc.tile_pool(name="psy", bufs=4, space="PSUM"))

        # PE warm-up: junk matmuls with no DMA deps run during the input
        # prologue so the HAM clock is at full rate when real work arrives
        wu = const_pool.tile([128, MOVW], dt.float16, name="wu")
        nc.vector.memset(wu[:, :], 0.0)
        ps_w = psh_pool.tile([128, MOVW], dt.float32, tag="psh")
        NWU = 20
        for k in range(NWU):
            nc.tensor.matmul(ps_w[:, :], wu[:, 0:128], wu[:, :],
                             start=(k == 0), stop=(k == NWU - 1))

        # process the smallest band first (shortest input prologue) and the
        # second-smallest last (shortest output tail)
        slot_order = [0] + list(range(NBANDS - 1, 0, -1))
        for si, s in enumerate(slot_order):
            cap = caps[s]
            tiles = [(o, min(MOVW, cap - o)) for o in range(0, cap, MOVW)]

            xg = xg_pool.tile([128, CC, cap], dt.float16, tag=f"xg{s}",
                              name=f"xg{s}")
            if si == 0:
                # head-split: a 128-token stripe lands first so the PE can
                # start while the rest of the panel streams in
                nc.sync.dma_start(
                    xg[:, :, 0:128],
                    xgd[s][:, 0:CC * 128]
                    .rearrange("p (cc t) -> p cc t", t=128))
                nc.sync.dma_start(
                    xg[:, :, 128:cap],
                    xgd[s][:, CC * 128:]
                    .rearrange("p (cc t) -> p cc t", t=cap - 128))
            else:
                nc.sync.dma_start(xg.rearrange("p cc t -> p (cc t)"),
                                  xgd[s][:, :])
            b1s = const_pool.tile([128, H // 128], dt.float32, tag=f"b1{s}",
                                  name=f"b1{s}")

            yT = yacc_pool.tile([128, CC, cap], dt.float32, tag=f"yT{s}",
                                name=f"yT{s}")

            for hc in range(NHC):
                w1c = w_pool.tile([128, HT * CC * 128], dt.float16, tag="w1c")
                for ht in range(HT):
                    nc.sync.dma_start(
                        w1c[:, ht * CC * 128:(ht + 1) * CC * 128],
                        w1[s, hc, ht, :, :])
                    if hc == 0 and ht == 0:
                        nc.sync.dma_start(b1s[:, :], b1[s, :, :])
                w2c = w_pool.tile([128, HT * C], dt.float16, tag="w2c")
                nc.sync.dma_start(w2c[:, :], w2[s, hc, :, :])

                htiles = tiles
                if si == 0 and hc == 0:
                    htiles = [(0, 128)] + [(o, min(MOVW, cap - o))
                                           for o in range(128, cap, MOVW)]
                hT = h_pool.tile([128, HT, cap], dt.float16, tag="hT")
                # h = relu(x @ w1c + b1): tile-outer so the last relu is off
                # the PE critical path when the y-phase starts
                for off, wd in htiles:
                    for ht in range(HT):
                        ps_h = psh_pool.tile([128, MOVW], dt.float32, tag="psh")
                        for cc in range(CC):
                            nc.tensor.matmul(
                                ps_h[:, 0:wd],
                                w1c[:, ht * CC * 128 + cc * 128:
                                    ht * CC * 128 + (cc + 1) * 128],
                                xg[:, cc, off:off + wd],
                                start=(cc == 0), stop=(cc == CC - 1))
                        nc.scalar.activation(
                            hT[:, ht, off:off + wd], ps_h[:, 0:wd],
                            AF.Relu,
                            bias=b1s[:, hc * HT + ht:hc * HT + ht + 1])
                # y += h @ w2c: ct-outer on the last chunk so each finished
                # output strip DMAs out while the rest still computes
                for ct in range(CC):
                    for off, wd in tiles:
                        ps_y = psy_pool.tile([128, MOVW], dt.float32, tag="psy")
                        for ht in range(HT):
                            nc.tensor.matmul(
                                ps_y[:, 0:wd],
                                w2c[:, ht * C + ct * 128:ht * C + (ct + 1) * 128],
                                hT[:, ht, off:off + wd],
                                start=(ht == 0), stop=(ht == HT - 1))
                        if hc == 0:
                            nc.vector.tensor_copy(yT[:, ct, off:off + wd],
                                                  ps_y[:, 0:wd])
                        else:
                            nc.vector.tensor_add(
                                yT[:, ct, off:off + wd],
                                yT[:, ct, off:off + wd], ps_y[:, 0:wd])
                        if hc == NHC - 1:
                            # stream each finished strip piece; the small
                            # remainder tile goes last -> shortest tail
                            nc.sync.dma_start(
                                ytd[s][ct, :, off:off + wd],
                                yT[:, ct, off:off + wd])

    nc.compile()
    return nc


def _route_host(x, w_router):
    """Exact top-2 routing on host (fp64; reference fp32 gap ~1e-5)."""
    xf = np.ascontiguousarray(np.asarray(x, dtype=np.float64).reshape(N, C))
    wr = np.asarray(w_router, dtype=np.float64)
    logits = xf @ wr                                     # [N, E]
    sel = np.argpartition(logits, E - 2, axis=1)[:, -2:]  # top2, unordered
    lv = np.take_along_axis(logits, sel, axis=1)
    swap = lv[:, 0] < lv[:, 1]
    sel[swap] = sel[swap][:, ::-1]
    lv[swap] = lv[swap][:, ::-1]
    # softmax over the two logits
    d = np.exp(lv[:, 1] - lv[:, 0])
    p0 = 1.0 / (1.0 + d)
    probs = np.stack([p0, 1.0 - p0], axis=1).astype(np.float32)  # [N, 2]
    return sel.astype(np.int64), probs


_PLAN_CACHE = {}


def _band_plan(counts):
    """Split each expert's token count into 2 groups binned into NBANDS
    bands of NCORES groups; minimize total band capacities (greedy-checked
    capacity search). Returns (caps ascending, plan) where
    plan[band][core] = (expert, start, size)."""
    key = counts.tobytes()
    if key in _PLAN_CACHE:
        return _PLAN_CACHE[key]
    order = np.argsort(-counts, kind="stable")

    pair_types = list(combinations_with_replacement(range(NBANDS), 2))
    scnt = np.sort(counts)[::-1]

    def prefix_ok(v):
        # necessary: sum of top-m counts <= max achievable sum of m pair
        # capacities (greedy pair picking under NCORES-per-band loads)
        rem = [NCORES] * NBANDS
        acc = 0
        m = 0
        for _ in range(len(scnt)):
            best = -1
            bi = bj = -1
            for i, j in pair_types:
                need = 2 if i == j else 1
                if rem[i] < need or (i != j and rem[j] < 1):
                    continue
                if v[i] + v[j] > best:
                    best = v[i] + v[j]
                    bi, bj = i, j
            if best < 0:
                return False
            rem[bi] -= 1
            rem[bj] -= 1
            acc += best
            if scnt[:m + 1].sum() > acc:
                return False
            m += 1
        return True

    from functools import lru_cache

    @lru_cache(maxsize=None)
    def _maxpairs(allowed_mask):
        # max #pairs under NCORES-per-band loads using only allowed types
        types = tuple(pair_types[t] for t in range(len(pair_types))
                      if (allowed_mask >> t) & 1)

        @lru_cache(maxsize=None)
        def rec(loads):
            best = 0
            for i, j in types:
                ld = list(loads)
                ld[i] += 1
                ld[j] += 1
                if ld[i] <= NCORES and ld[j] <= NCORES:
                    best = max(best, 1 + rec(tuple(ld)))
            return best

        return rec((0,) * NBANDS)

    def threshold_ok(v):
        # for each m: the m largest experts need m pairs with sum >= c_m
        for m in range(1, len(scnt) + 1):
            c = int(scnt[m - 1])
            mask = 0
            for t, (i, j) in enumerate(pair_types):
                if v[i] + v[j] >= c:
                    mask |= 1 << t
            if mask == 0 or _maxpairs(mask) < m:
                return False
        return True

    def assign(v):
        if not prefix_ok(v) or not threshold_ok(v):
            return None
        # reconstruct one concrete assignment (dict DP with parents)
        layers = [{(0,) * NBANDS: None}]
        for e in order:
            c = int(counts[e])
            nxt = {}
            for st in layers[-1]:
                for i, j in pair_types:
                    if v[i] + v[j] < c:
                        continue
                    ld = list(st)
                    ld[i] += 1
                    ld[j] += 1
                    if ld[i] > NCORES or ld[j] > NCORES:
                        continue
                    key = tuple(ld)
                    if key not in nxt:
                        nxt[key] = (st, (i, j))
            if not nxt:
                return None
            layers.append(nxt)
        st = next(iter(layers[-1]))
        out = []
        for k in range(len(order) - 1, -1, -1):
            prev, ij = layers[k + 1][st]
            out.append((order[k], ij[0], ij[1]))
            st = prev
        return out[::-1]

    # capacities stay multiples of 16: a 16-misaligned moving operand costs
    # ~25% extra PE time per matmul (SBUF line granularity). Search totals
    # ascending from the per-core floor; first feasible partition wins.
    u = 16
    cmax = int(np.ceil(counts.max() / u))
    s0 = int(np.ceil(counts.sum() / NCORES / u))
    best = None
    s = s0
    while best is None:
        for v1 in range(min(cmax, s - 3), (cmax + 1) // 2 - 1, -1):
            for v2 in range(min(v1, s - v1 - 2), 0, -1):
                if v1 + v2 < cmax:
                    break
                for v3 in range(min(v2, s - v1 - v2 - 1), 0, -1):
                    v4 = s - v1 - v2 - v3
                    if v4 < 1 or v4 > v3:
                        continue
                    cand = (v1 * u, v2 * u, v3 * u, v4 * u)
                    if assign(cand) is not None:
                        best = cand
                        break
                if best:
                    break
            if best:
                break
        s += 1
    v = tuple(sorted(best))               # ascending caps
    asg = assign(tuple(sorted(v, reverse=True)))
    # map band index of the search (desc order) to ascending slot index
    remap = {i: NBANDS - 1 - i for i in range(NBANDS)}
    plan = [[None] * NCORES for _ in range(NBANDS)]
    fill = [0] * NBANDS
    for e, i, j in asg:
        bi, bj = remap[i], remap[j]
        c = int(counts[e])
        gj = min(v[bj], c)
        gi = c - gj
        for b, start, size in ((bj, 0, gj), (bi, gj, gi)):
            plan[b][fill[b]] = (e, start, size)
            fill[b] += 1
    for b in range(NBANDS):
        while fill[b] < NCORES:
            plan[b][fill[b]] = (0, 0, 0)
            fill[b] += 1
    _PLAN_CACHE[key] = (v, plan)
    return v, plan


def prepare_in_maps(x, w_router, w1, b1, w2, b2):
    x = np.asarray(x, dtype=np.float32)
    w1 = np.asarray(w1, dtype=np.float32)
    b1 = np.asarray(b1, dtype=np.float32)
    w2 = np.asarray(w2, dtype=np.float32)

    sel, probs = _route_host(x, w_router)

    # per-expert compact token lists + gates
    flat_e = sel.ravel()                       # [2N] expert ids
    flat_t = np.repeat(np.arange(N), 2)        # token ids
    flat_g = probs.ravel()
    order = np.argsort(flat_e, kind="stable")
    counts = np.bincount(flat_e, minlength=E)
    starts = np.concatenate([[0], np.cumsum(counts)])
    tok_by_e = [flat_t[order[starts[e]:starts[e + 1]]] for e in range(E)]
    gate_by_e = [flat_g[order[starts[e]:starts[e + 1]]] for e in range(E)]

    caps, plan = _band_plan(counts)

    xf16T = np.ascontiguousarray(
        x.reshape(N, C).T.astype(np.float16))      # [C, N]
    # partition-major weight panels: every DMA row is contiguous in HBM
    # w1p[e, hc, p, cc*HC+h'] = w1[e, cc*128+p, hc*HC+h']
    w1p = np.ascontiguousarray(
        w1.astype(np.float16).reshape(E, CC, 128, NHC, HT, 128)
        .transpose(0, 3, 4, 2, 1, 5).reshape(E, NHC, HT, 128, CC * 128))
    # w2p[e, hc, p, ht*C+ck] = w2[e, hc*HC+ht*128+p, ck]
    w2p = np.ascontiguousarray(
        w2.astype(np.float16).reshape(E, NHC, HT, 128, C)
        .transpose(0, 1, 3, 2, 4).reshape(E, NHC, 128, HT * C))
    # b1p[e, p, htg] = b1[e, htg*128+p]
    b1p = np.ascontiguousarray(
        b1.reshape(E, H // 128, 128).transpose(0, 2, 1))

    in_maps = []
    for c in range(NCORES):
        ex = [plan[s][c][0] for s in range(NBANDS)]
        im = {
            "w1": np.ascontiguousarray(w1p[ex]),
            "w2": np.ascontiguousarray(w2p[ex]),
            "b1": np.ascontiguousarray(b1p[ex]),
        }
        for s in range(NBANDS):
            e, g0, gn = plan[s][c]
            idx = tok_by_e[e][g0:g0 + gn]
            full = np.concatenate(
                [idx, np.zeros(caps[s] - gn, dtype=np.int64)])
            g3 = xf16T[:, full].reshape(CC, 128, caps[s])
            if s == 0:
                # sectioned: 128-token head stripe first, then the rest
                pa = g3[:, :, 0:128].transpose(1, 0, 2).reshape(128, -1)
                pb = g3[:, :, 128:].transpose(1, 0, 2).reshape(128, -1)
                im[f"xg{s}"] = np.ascontiguousarray(
                    np.concatenate([pa, pb], axis=1))
            else:
                # xg[p, cc*cap+t] = x[tok_t, cc*128+p]
                im[f"xg{s}"] = np.ascontiguousarray(
                    g3.transpose(1, 0, 2).reshape(128, CC * caps[s]))
        in_maps.append(im)

    _LAST_META.update(dict(caps=caps, plan=plan, tok_by_e=tok_by_e,
                           gate_by_e=gate_by_e, counts=counts))
    if caps not in _NC_CACHE:
        _NC_CACHE[caps] = _build(caps)
    _NC_CACHE["nc"] = _NC_CACHE[caps]
    return in_maps


def combine(results, b2):
    m = _LAST_META
    b2 = np.asarray(b2, dtype=np.float32)
    out = np.zeros((N, C), dtype=np.float32)
    for c in range(NCORES):
        r = results[c]
        for s in range(NBANDS):
            e, g0, gn = m["plan"][s][c]
            if gn == 0:
                continue
            idx = m["tok_by_e"][e][g0:g0 + gn]
            g = m["gate_by_e"][e][g0:g0 + gn]
            # y[tok_slot, ct*128+p] = yt[ct, p, slot]
            y = r[f"yt{s}"].transpose(2, 0, 1).reshape(m["caps"][s], C)[:gn]
            # tokens unique within one expert group -> fancy-index add
            out[idx] += (y + b2[e][None, :]) * g[:, None]
    return out.reshape(B, T, C)


def kernel(x, w_router, w1, b1, w2, b2):
    in_maps = prepare_in_maps(x, w_router, w1, b1, w2, b2)
    nc = _NC_CACHE["nc"]
    res = bass_utils.run_bass_kernel_spmd(nc, in_maps, core_ids=list(range(NCORES)))
    kernel.last_results = res
    return combine(res.results, np.asarray(b2, dtype=np.float32))
